# revision 1
# baseline (speedup 1.0000x reference)
"""Multi-head attention (B=1, L=4096, D=1024, H=16, d_k=64) on 8 TRN2 NeuronCores.

Sharding: head/tensor parallel. Core c owns heads 2c, 2c+1 (128 dmodel dims):
its slices of the QKV projection weights, full-L attention for its 2 heads,
and the partial O-projection for its 128-dim slice. Host sums the 8 partial
outputs (the allreduce of row-sharded tensor parallelism, done at gather).

On-chip layout is fully transposed ([feature, L]) so the softmax reductions
become PE matmuls (a ones-column appended to V yields the softmax denominator)
and no on-chip transposes of big activations are needed. All matmuls run in
f32r (TF32) at full PE rate; softmax itself is exact fp32 (scores ~N(0,1) so
exp() without max-subtraction is safe).

Activations/outputs cross HBM pre-tiled as [t, qc, 128, 512] contiguous blocks
so every DMA is a single 256 KiB contiguous transfer (one descriptor), and
phase-1 loads alternate between the two HWDGE queues (SP + ACT).
"""
import sys
if '/opt/trn_rl_repo' not in sys.path:
    sys.path.insert(0, '/opt/trn_rl_repo')

import os
import numpy as np
from contextlib import ExitStack

import concourse.bass as bass
import concourse.tile as tile
from concourse import bacc, mybir
from concourse._compat import with_exitstack
from concourse.bass_utils import run_bass_kernel_spmd
from concourse.masks import make_identity

F32 = mybir.dt.float32
F32R = mybir.dt.float32r
F16 = mybir.dt.float16
AF = mybir.ActivationFunctionType

N_CORES = 8
L = 4096
D = 1024
QC = 512                 # q-chunk (free dim of ST/AT/OT)
NQC = L // QC            # 8
NKT = L // 128           # 32 k-tiles
NDT = D // 128           # 8 dmodel tiles
GK = 2                   # k-tiles per exp group
SCALE = 0.125            # 1/sqrt(d_k)


@with_exitstack
def _mha_core_kernel(ctx, tc, outs, ins, reps=1):
    sb = ctx.enter_context(tc.tile_pool(name="sb", bufs=1))
    phases = os.environ.get("MHA_PHASES", "12")
    for _rep in range(reps):
        _mha_body(tc, sb, outs, ins, phases=phases)


def _mha_body(tc, sb, outs, ins, phases="12"):
    nc = tc.nc
    (outT,) = outs           # [NDT, NQC, 128, QC]
    qT, kT, vT, wq, wk, wv, wo = ins
    # qT/kT/vT: [NDT, NQC, 128, QC] pre-tiled transposed activations
    # wq/wk/wv: [128, NDT, 128]  (lhsT per ktile);  wo: [64, 2, D]

    # ---- weights (single contiguous DMAs) ----
    wq_s = sb.tile([128, NDT, 128], F16, tag="wq")
    wk_s = sb.tile([128, NDT, 128], F16, tag="wk")
    wv_s = sb.tile([128, NDT, 128], F16, tag="wv")
    for w_s, w_d in ((wq_s, wq), (wk_s, wk), (wv_s, wv)):
        nc.sync.dma_start(w_s[:], w_d[:])
    wo_s = sb.tile([64, 2, D], F32R, tag="wo")
    nc.sync.dma_start(wo_s[:], wo[:].bitcast(F32R))

    ident = sb.tile([128, 128], F32, tag="ident")
    make_identity(nc, ident[:])

    # ---- persistent activations ----
    QT_s = sb.tile([128, L], F32R, tag="QT")
    KT_s = sb.tile([128, L], F32R, tag="KT")
    VT_s = sb.tile([128, L], F32, tag="VT")
    Vaug = sb.tile([128, NKT, 130], F32R, tag="Vaug")
    ones_s = sb.tile([128, NKT], F32, tag="ones")
    nc.vector.memset(ones_s[:], 1.0)
    nc.vector.tensor_copy(Vaug[:, :, 64], ones_s[:])
    nc.vector.tensor_copy(Vaug[:, :, 129], ones_s[:])

    # ========== phase 1: K, Q, V projections (t-major 2 MiB streams) ==========
    if "1" in phases:
      with (
        tc.tile_pool(name="xblk", bufs=3) as xblk,
        tc.tile_pool(name="pacc", bufs=1, space="PSUM") as pacc,
      ):
        def proj(dst, w_s, src_d):
            accs = [pacc.tile([128, QC], F32, tag=f"acc{qc}", name=f"acc{qc}")
                    for qc in range(NQC)]
            for t in range(NDT):
                blk = xblk.tile([128, L], F16, tag="blk")
                nc.sync.dma_start(blk[:], src_d[t * 128:(t + 1) * 128, :])
                for qc in range(NQC):
                    nc.tensor.matmul(accs[qc][:], w_s[:, t, :],
                                     blk[:, qc * QC:(qc + 1) * QC],
                                     start=(t == 0), stop=(t == NDT - 1))
            for qc in range(NQC):
                nc.vector.tensor_copy(dst[:, qc * QC:(qc + 1) * QC], accs[qc][:])

        proj(KT_s, wk_s, kT)
        proj(VT_s, wv_s, vT)
        proj(QT_s, wq_s, qT)
      # transpose VT -> Vaug rows (L-major), heads split around ones columns
      with tc.tile_pool(name="tps", bufs=2, space="PSUM") as tps:
        for rt in range(NKT):
            tp = tps.tile([128, 128], F32, tag="tp")
            nc.tensor.transpose(tp[:], VT_s[:, rt * 128:(rt + 1) * 128], ident[:])
            nc.vector.tensor_copy(Vaug[:, rt, 0:64], tp[:, 0:64])
            nc.vector.tensor_copy(Vaug[:, rt, 65:129], tp[:, 64:128])

    # ====== phase 2: per q-chunk: attention + O-proj ======
    if "2" in phases:
      with (
        tc.tile_pool(name="st0", bufs=1, space="PSUM") as pst0,
        tc.tile_pool(name="st1", bufs=1, space="PSUM") as pst1,
        tc.tile_pool(name="pot", bufs=1, space="PSUM") as pot,
        tc.tile_pool(name="pop", bufs=1, space="PSUM") as pop,
        tc.tile_pool(name="pat", bufs=2) as pat,
        tc.tile_pool(name="psm", bufs=2) as psm,
        tc.tile_pool(name="poc", bufs=2) as poc,
      ):
        for qc in range(NQC):
            q0, q1 = qc * QC, (qc + 1) * QC
            # -- attention (scores transposed: [k, q])
            ot0 = pot.tile([65, QC], F32, tag="ot0")
            ot1 = pot.tile([65, QC], F32, tag="ot1")
            for g in range(NKT // GK):
                st0 = pst0.tile([128, GK * QC], F32, tag="st0")
                st1 = pst1.tile([128, GK * QC], F32, tag="st1")
                for j in range(GK):
                    kt = g * GK + j
                    nc.tensor.matmul(st0[:, j * QC:(j + 1) * QC],
                                     KT_s[0:64, kt * 128:(kt + 1) * 128],
                                     QT_s[0:64, q0:q1], start=True, stop=True)
                    nc.tensor.matmul(st1[:, j * QC:(j + 1) * QC],
                                     KT_s[64:128, kt * 128:(kt + 1) * 128],
                                     QT_s[64:128, q0:q1], start=True, stop=True)
                at0 = pat.tile([128, GK * QC], F32R, tag="at0")
                nc.scalar.activation(at0[:], st0[:], AF.Exp, scale=SCALE)
                at1 = pat.tile([128, GK * QC], F32R, tag="at1")
                nc.scalar.activation(at1[:], st1[:], AF.Exp, scale=SCALE)
                for j in range(GK):
                    kt = g * GK + j
                    nc.tensor.matmul(ot0[:], Vaug[:, kt, 0:65],
                                     at0[:, j * QC:(j + 1) * QC],
                                     start=(kt == 0), stop=(kt == NKT - 1))
                    nc.tensor.matmul(ot1[:], Vaug[:, kt, 65:130],
                                     at1[:, j * QC:(j + 1) * QC],
                                     start=(kt == 0), stop=(kt == NKT - 1))

            # -- normalize by the rowsum (row 64 of OT)
            otn = []
            for h, ot in ((0, ot0), (1, ot1)):
                rec = psm.tile([1, QC], F32, tag=f"rec{h}")
                nc.vector.reciprocal(rec[:], ot[64:65, :])
                bc = psm.tile([64, QC], F32, tag=f"bc{h}")
                nc.gpsimd.partition_broadcast(bc[:], rec[:])
                on = psm.tile([64, QC], F32R, tag=f"otn{h}")
                nc.vector.tensor_mul(on[:], ot[0:64, :], bc[:])
                otn.append(on)

            # -- O-projection (contract this core's 128 dmodel dims)
            for mt in range(NDT):
                op = pop.tile([128, QC], F32, tag="op")
                nc.tensor.matmul(op[:], wo_s[:, 0, mt * 128:(mt + 1) * 128],
                                 otn[0][:], start=True, stop=False)
                nc.tensor.matmul(op[:], wo_s[:, 1, mt * 128:(mt + 1) * 128],
                                 otn[1][:], start=False, stop=True)
                oc = poc.tile([128, QC], F32, tag="oc")
                nc.vector.tensor_copy(oc[:], op[:])
                nc.sync.dma_start(outT[mt, qc, :, :], oc[:])


_PROGRAM = None


def _declare_io(nc):
    mk = lambda n, s, kind, dt=F32: nc.dram_tensor(n, list(s), dt, kind=kind).ap()
    blk4 = (NDT, NQC, 128, QC)
    ins = [mk("qT", (D, L), "ExternalInput", F16), mk("kT", (D, L), "ExternalInput", F16),
           mk("vT", (D, L), "ExternalInput", F16),
           mk("wq", (128, NDT, 128), "ExternalInput", F16),
           mk("wk", (128, NDT, 128), "ExternalInput", F16),
           mk("wv", (128, NDT, 128), "ExternalInput", F16),
           mk("wo", (64, 2, D), "ExternalInput")]
    outs = [mk("outT", blk4, "ExternalOutput")]
    return ins, outs


def _build_program(reps=1):
    global _PROGRAM
    if _PROGRAM is not None and reps == 1:
        return _PROGRAM
    nc = bacc.Bacc("TRN2", target_bir_lowering=False, debug=False,
                   num_devices=N_CORES)
    ins, outs = _declare_io(nc)
    with tile.TileContext(nc) as tc:
        _mha_core_kernel(tc, outs, ins, reps=reps)
    nc.compile()
    if reps == 1:
        _PROGRAM = nc
    return nc


def _tile_T(x):
    """[L, D] -> transposed [D, L] contiguous fp16."""
    return np.ascontiguousarray(x.T.astype(np.float16))


def _tile_w(w_slice):
    """[128, D] (rows = this core's dims) -> lhsT layout [128, NDT, 128]."""
    # lhsT[p, t, m] = w_slice[m, t*128+p]
    return np.ascontiguousarray(
        w_slice.reshape(128, NDT, 128).transpose(2, 1, 0).astype(np.float16))


def make_in_maps(query, key, value, w_q, w_k, w_v, w_o):
    qT = _tile_T(query.reshape(L, D))
    kT = _tile_T(key.reshape(L, D))
    vT = _tile_T(value.reshape(L, D))
    in_maps = []
    for c in range(N_CORES):
        sl = slice(c * 128, (c + 1) * 128)
        wo_c = w_o[:, sl]                       # [D, 128]
        wo_t = np.ascontiguousarray(
            wo_c.T.reshape(2, 64, D).transpose(1, 0, 2))   # [64, 2, D]
        in_maps.append({
            "qT": qT, "kT": kT, "vT": vT,
            "wq": _tile_w(w_q[sl]),
            "wk": _tile_w(w_k[sl]),
            "wv": _tile_w(w_v[sl]),
            "wo": wo_t,
        })
    return in_maps


def gather_out(results):
    """Sum per-core pre-tiled partials and restore [1, L, D]."""
    acc = results[0]["outT"].astype(np.float32).copy()
    for c in range(1, N_CORES):
        acc += results[c]["outT"]
    # acc[t, qc, p, j] = out.T[t*128+p, qc*512+j] = out[qc*512+j, t*128+p]
    out = acc.transpose(1, 3, 0, 2).reshape(L, D)
    return np.ascontiguousarray(out).reshape(1, L, D)


def run(in_maps, trace=False):
    nc = _build_program()
    return run_bass_kernel_spmd(nc, in_maps, core_ids=list(range(N_CORES)),
                                trace=trace)


def kernel(query, key, value, w_q, w_k, w_v, w_o):
    query = np.asarray(query, dtype=np.float32)
    key = np.asarray(key, dtype=np.float32)
    value = np.asarray(value, dtype=np.float32)
    w_q = np.asarray(w_q, dtype=np.float32)
    w_k = np.asarray(w_k, dtype=np.float32)
    w_v = np.asarray(w_v, dtype=np.float32)
    w_o = np.asarray(w_o, dtype=np.float32)

    res = run(make_in_maps(query, key, value, w_q, w_k, w_v, w_o))
    return gather_out(res.results)

